# revision 31
# baseline (speedup 1.0000x reference)
"""ChebConv (K=4) message-passing kernel for 8 Trainium2 NeuronCores.

Architecture (1D graph partitioning by destination node):
  - 50000 nodes split contiguously into 8 shards of 6250, each padded to
    6272 = 49 tiles of 128 destinations.
  - Per hop, each core gathers the source rows of its ~100k edges from a
    replicated table (dinv-prescaled: table_k[v] = dinv[v] * tx_k[v], rows
    padded to a 256B stride) using the custom InstDMAGatherAnt (dma_gather)
    SWDGE instruction.  Indices are int16, so the table is addressed as a
    low half (rows < 32768) and a high half via two calls with different
    base APs.
  - The edge stream is ordered (half, dest-tile, 64-dest window, dest); each
    128-slot chunk is segment-reduced into its window's PSUM accumulator with
    a TensorE matmul against a one-hot "staircase" matrix
    (stair[slot, d] = 1 iff slot's dest-within-window == d), generated on the
    vector engine from iota==destvec.  Chunk padding slots have destvec -1.
  - Chebyshev recurrence tx_k = -2*dinv*red - tx_{k-2} on the vector engine;
    next hop's table rebuilt via a packed [npc, 96] AllGather of the rescaled
    shard plus a local restride DMA into the 256B-row-stride table.
  - out = sum_k tx_k @ W_k + bias via PE transposes + matmuls, written
    feature-major; the host strips padding.
"""

import os
import sys

for _p in ("/opt/trn_rl_repo", "/root/.axon_site/_ro/trn_rl_repo"):
    if os.path.isdir(_p) and _p not in sys.path:
        sys.path.insert(0, _p)
        break

import numpy as np

import concourse.bacc as bacc
import concourse.bass as bass
import concourse.mybir as mybir
import concourse.tile as tile
from concourse import bass_utils

F32 = mybir.dt.float32
BF16 = mybir.dt.bfloat16
I16 = mybir.dt.int16

USE_FP8 = True  # table/gather/stair dtype (halves AG + gather DMA bytes)
TDT = mybir.dt.float8e4 if USE_FP8 else BF16
NP_TDT = mybir.dt.np(TDT)
TSZ = 1 if USE_FP8 else 2

N_NODES = 50000
D = 96
DPAD = 256 // TSZ  # table row = 256B stride, in elements
K_HOPS = 4
N_CORES = 8
P = 128
LOBASE = 32768
CALL_CHUNKS = 8  # chunks per dma_gather call (1024 idxs — ucode ring cap;
# >1024-idx calls hang the device regardless of dynamic_dma_scratch_size)
DMA_SCRATCH = 65536  # headroom for 4 SWDGE queue rings
N_QUEUES = 4  # round-robin gather calls across SWDGE queues (~2.5x faster)
W = 128  # staircase window width (dests per psum accumulation group)
STAIR_BATCH = 32  # chunks per staircase-generation op


def _plan_sizes(n_nodes, n_cores):
    npc_raw = n_nodes // n_cores
    assert npc_raw * n_cores == n_nodes
    n_tiles = -(-npc_raw // P)
    npc = n_tiles * P
    return npc_raw, npc, n_tiles


def preprocess(x, edge_index, n_nodes, n_cores):
    npc_raw, npc, n_tiles = _plan_sizes(n_nodes, n_cores)
    npt = npc * n_cores
    n_pad = npc - npc_raw

    row = np.asarray(edge_index[0], dtype=np.int64)
    col = np.asarray(edge_index[1], dtype=np.int64)
    deg = np.bincount(row, minlength=n_nodes).astype(np.int64)
    dinv = np.zeros(n_nodes, dtype=np.float32)
    nz = deg > 0
    dinv[nz] = (1.0 / np.sqrt(deg[nz].astype(np.float64))).astype(np.float32)

    # pad-only remap: node v -> v + n_pad * (v // npc_raw)
    blk = np.arange(n_nodes) // npc_raw
    new_id = np.arange(n_nodes) + n_pad * blk

    x_new = np.zeros((npt, D), dtype=np.float32)
    x_new[new_id] = np.asarray(x, dtype=np.float32)
    dinv_new = np.zeros(npt, dtype=np.float32)
    dinv_new[new_id] = dinv

    table0 = np.zeros((npt, DPAD), dtype=NP_TDT)
    table0[:, :D] = (x_new * dinv_new[:, None]).astype(NP_TDT)

    row_new = new_id[row]
    col_new = new_id[col]
    core_of_edge = row_new // npc

    # ---- global chunk schedule (same for all cores) ----
    # chunks are grouped by (tile, half, 64-dest window) so the staircase
    # matrices are only 64 wide
    d_loc_all = row_new % npc
    t_all = d_loc_all // P
    w_all = (d_loc_all % P) // W  # window within tile
    h_all = (col_new >= LOBASE).astype(np.int64)
    nw = P // W
    counts = np.zeros((n_cores, n_tiles, 2, nw), dtype=np.int64)
    np.add.at(counts, (core_of_edge, t_all, h_all, w_all), 1)
    n_ch = -(-counts.max(axis=0) // P)  # [n_tiles, 2, nw]

    # stream order: all lo (t, w ascending), then all hi
    chunk_base = np.zeros((n_tiles, 2, nw), dtype=np.int64)
    pos = 0
    order_th = [(t, 0, w) for t in range(n_tiles) for w in range(nw)] + [
        (t, 1, w) for t in range(n_tiles) for w in range(nw)
    ]
    chunk_ranges = {}
    for t, h, w in order_th:
        chunk_base[t, h, w] = pos
        chunk_ranges[(t, h, w)] = (pos, pos + int(n_ch[t, h, w]))
        pos += int(n_ch[t, h, w])
    n_chunks = pos
    n_lo_chunks = int(n_ch[:, 0, :].sum())
    S = n_chunks * P  # total slots

    # call plan: contiguous chunk runs, single half, <= CALL_CHUNKS
    calls = []  # (half, chunk_start, n_chunks)
    for h, lo_, hi_ in ((0, 0, n_lo_chunks), (1, n_lo_chunks, n_chunks)):
        c0 = lo_
        while c0 < hi_:
            n = min(CALL_CHUNKS, hi_ - c0)
            calls.append((h, c0, n))
            c0 += n

    # ---- per-core streams ----
    idxw_all = []
    destvec_all = []
    x_shards = []
    dinv_pm = []
    for c in range(n_cores):
        m = core_of_edge == c
        d_loc = d_loc_all[m]
        hh = h_all[m]
        cn = col_new[m]
        nw = P // W
        g_un = (hh * n_tiles + d_loc // P) * nw + (d_loc % P) // W
        order = np.lexsort((d_loc, g_un))
        d_s = d_loc[order]
        h_s = hh[order]
        c_s = cn[order] - h_s * LOBASE
        t_s = d_s // P
        w_s = (d_s % P) // W
        g_s = g_un[order]  # group id in stream order
        gcnt = np.bincount(g_s, minlength=2 * n_tiles * nw)
        gstart = np.concatenate([[0], np.cumsum(gcnt)])[:-1]
        pos_in_g = np.arange(len(d_s)) - gstart[g_s]
        base_slots = chunk_base[t_s, h_s, w_s] * P
        slot = base_slots + pos_in_g

        idx_stream = np.zeros(S, dtype=np.int16)
        destvec = np.full(S, -1.0, dtype=mybir.dt.np(BF16))
        idx_stream[slot] = c_s.astype(np.int16)
        destvec[slot] = (d_s % W).astype(mybir.dt.np(BF16))

        # wrapped idx layout for dma_gather: [128, S//16], replicated per 16
        w16 = idx_stream.reshape(S // 16, 16).T  # [16, S//16]
        idxw = np.tile(w16, (8, 1))  # [128, S//16]
        idxw_all.append(np.ascontiguousarray(idxw))
        # destvec partition-major per chunk: [128, n_chunks]
        destvec_all.append(
            np.ascontiguousarray(destvec.reshape(n_chunks, P).T)
        )
        x_shards.append(np.ascontiguousarray(x_new[c * npc : (c + 1) * npc]))
        dinv_pm.append(
            np.ascontiguousarray(
                dinv_new[c * npc : (c + 1) * npc].reshape(n_tiles, P).T
            )
        )

    meta = dict(
        npc_raw=npc_raw,
        npc=npc,
        n_tiles=n_tiles,
        npt=npt,
        S=S,
        n_chunks=n_chunks,
        n_lo_chunks=n_lo_chunks,
        chunk_ranges=chunk_ranges,
        calls=calls,
        new_id=new_id,
    )
    return meta, table0, x_shards, dinv_pm, idxw_all, destvec_all


CC_ENGINE = "gpsimd"  # walrus' checkValidEngines only allows Pool for ccs
# "copy":     AG full-width [npt, DPAD] Shared packed, plain copy to Local
# "direct":   AG full-width Shared packed IS the gather table (no copy)
# "packed96": AG [npt, D] packed (2.7x fewer cc bytes), strided restride
TABLE_MODE = os.environ.get("TABLE_MODE", "direct")


def _cc_allgather(nc, rg, in_ap, out_ap):
    eng = getattr(nc, CC_ENGINE)
    bass.BassGpSimd.collective_compute(
        eng,
        "AllGather",
        mybir.AluOpType.bypass,
        replica_groups=rg,
        ins=[in_ap],
        outs=[out_ap],
    )


def _dma_gather_rows(g, out_ap, in_ap, idxs_ap, num_idxs, queue_num=0):
    """bass.dma_gather minus the %256 payload assert (non-transpose HBM
    path): gathers D elems (D*TSZ bytes) per index from 256B-strided rows."""
    import concourse.ap_utils as ap_utils

    elem_size, elem_step = D, DPAD
    assert idxs_ap.dtype == I16
    assert in_ap.ap[0][0] == elem_step
    assert in_ap.ap[-1][1] == out_ap.ap[-1][1] == elem_size
    assert ap_utils.ap_is_contiguous(out_ap.ap[1:])
    assert ap_utils.ap_is_contiguous(idxs_ap.ap[1:])
    assert out_ap.ap[0][1] * out_ap.ap[1][1] == num_idxs
    stride_bytes_256 = (elem_step * TSZ) // 256
    _in_ap = g.lower_ap_dma(in_ap, for_custom_bir_dma=True)
    _idxs_ap = g.lower_ap(idxs_ap)
    _out_ap = g.lower_ap(out_ap)
    return g.add_instruction(
        mybir.InstDMAGatherAnt(
            name=g.bass.get_next_instruction_name(),
            ins=[*_in_ap, _idxs_ap, g.lower_val_access(g.to_reg(num_idxs))],
            outs=[_out_ap],
            transpose=False,
            num_idxs=num_idxs,
            elem_size=elem_size,
            stride_bytes_256=stride_bytes_256,
            gen_mode=0,
            single_packet=True,
            queue_num=queue_num,
            sbuf_tokens_per_rank=0,
            sbuf_free_dim_per_rank=0,
            sbuf_free_dim_pad_per_rank=0,
            sbuf_byte_offset=0,
        )
    )


def build_program(meta, n_cores, repeat=1, ablate=frozenset()):
    npc = meta["npc"]
    n_tiles = meta["n_tiles"]
    npt = meta["npt"]
    S = meta["S"]
    n_chunks = meta["n_chunks"]
    chunk_ranges = meta["chunk_ranges"]
    calls = meta["calls"]
    lo_rows = min(LOBASE, npt)

    nc = bacc.Bacc(
        "TRN2",
        target_bir_lowering=False,
        debug=False,
        num_devices=n_cores,
        dynamic_dma_scratch_size=DMA_SCRATCH,
        num_swdge_queues=N_QUEUES,
    )
    t_table0 = nc.dram_tensor("table0", [npt, DPAD], TDT, kind="ExternalInput")
    t_x = nc.dram_tensor("x", [npc, D], F32, kind="ExternalInput")
    t_dinv = nc.dram_tensor("dinv", [P, n_tiles], F32, kind="ExternalInput")
    t_idxw = nc.dram_tensor("idxw", [P, S // 16], I16, kind="ExternalInput")
    t_dv = nc.dram_tensor("destvec", [P, n_chunks], BF16, kind="ExternalInput")
    t_w = nc.dram_tensor("w", [K_HOPS, D, D], F32, kind="ExternalInput")
    t_b = nc.dram_tensor("bias", [D], F32, kind="ExternalInput")
    t_out = nc.dram_tensor("outT", [D, npc], BF16, kind="ExternalOutput")

    rg = [list(range(n_cores))]

    with tile.TileContext(nc) as tc:
        with (
            tc.tile_pool(name="persist", bufs=1) as sb,
            tc.tile_pool(name="gather", bufs=3) as gp,
            tc.tile_pool(name="stair", bufs=3) as stp,
            tc.tile_pool(name="work", bufs=3) as wp,
            tc.tile_pool(name="dram", bufs=1, space="DRAM") as dp,
            tc.tile_pool(name="psum", bufs=1, space="PSUM") as pp,
        ):
            # ---- persistent loads ----
            idxw_sb = sb.tile([P, S // 16], I16)
            nc.sync.dma_start(out=idxw_sb[:], in_=t_idxw.ap())
            dv_sb = sb.tile([P, n_chunks], BF16)
            nc.sync.dma_start(out=dv_sb[:], in_=t_dv.ap())
            dinv_sb = sb.tile([P, n_tiles], F32)
            nc.sync.dma_start(out=dinv_sb[:], in_=t_dinv.ap())
            mdinv_sb = sb.tile([P, n_tiles], F32)
            nc.vector.tensor_scalar_mul(mdinv_sb[:], dinv_sb[:], -1.0)
            m2dinv_sb = sb.tile([P, n_tiles], F32)
            nc.vector.tensor_scalar_mul(m2dinv_sb[:], dinv_sb[:], -2.0)

            w_f32 = wp.tile([D, K_HOPS * D], F32, tag="wf")
            for k in range(K_HOPS):
                nc.sync.dma_start(out=w_f32[:, k * D : (k + 1) * D], in_=t_w.ap()[k])
            w_bf = sb.tile([D, K_HOPS * D], BF16)
            nc.vector.tensor_copy(w_bf[:], w_f32[:])
            bias_sb = sb.tile([D, 1], F32)
            nc.sync.dma_start(out=bias_sb[:], in_=t_b.ap()[:, None])

            from concourse.masks import make_identity

            ident = sb.tile([P, P], BF16)
            make_identity(nc, ident[:])

            iota_sb = sb.tile([P, P], BF16)
            nc.gpsimd.iota(
                iota_sb[:],
                pattern=[[1, P]],
                base=0,
                channel_multiplier=0,
                allow_small_or_imprecise_dtypes=True,
            )

            # tx buffers
            tx_bf = []
            for k in range(K_HOPS):
                txb = sb.tile([P, n_tiles, D], BF16, name=f"tx_bf{k}")
                tx_bf.append(txb)
            tx1_f = sb.tile([P, n_tiles, D], F32)
            acc_sb = sb.tile([P, n_tiles, D], F32)
            # h_all rows padded to the 256B table stride so the AllGather
            # lands directly in gatherable table layout (no restride DMA)
            h_all = sb.tile([P, n_tiles, DPAD], TDT, name="h_all")
            nc.vector.memset(h_all[:], 0.0)

            # x -> tx_bf[0] via acc_sb (acc_sb is scratch until hop 1)
            nc.sync.dma_start(
                out=acc_sb[:], in_=t_x.ap().rearrange("(t p) f -> p t f", p=P)
            )
            nc.vector.tensor_copy(tx_bf[0][:], acc_sb[:])

            # DRAM tables / bounce buffers (per repeat: Shared tiles allow
            # only a single writer).
            BW = D if TABLE_MODE == "packed96" else DPAD  # AG row width
            tables_r = []
            bounces_r = []
            packed_r = []
            for rep in range(repeat):
                tables = [t_table0.ap()]
                bounces = []
                packeds = []
                for k in range(1, K_HOPS - 1):
                    pk = dp.tile(
                        [npt, BW], TDT, addr_space="Shared",
                        name=f"packed{rep}_{k}",
                    )
                    if TABLE_MODE == "direct":
                        tables.append(pk[:])
                    else:
                        tb = dp.tile([npt, DPAD], TDT, name=f"table{rep}_{k}")
                        tables.append(tb[:])
                    bn = dp.tile([npc, BW], TDT, name=f"bounce{rep}_{k}")
                    bounces.append(bn)
                    packeds.append(pk)
                tables_r.append(tables)
                bounces_r.append(bounces)
                packed_r.append(packeds)

            # ---- hops ----
            for rep in range(repeat):
              tables = tables_r[rep]
              bounces = bounces_r[rep]
              packeds = packed_r[rep]
              idxw_use = idxw_sb
              if "serial" in ablate and rep > 0:
                  # chain rep r's gathers on rep r-1's final tx state so the
                  # repeat-slope measures single-invocation latency (~2us
                  # pollution per rep)
                  tok_f = wp.tile([P, 1], F32, tag="tok", name=f"tok{rep}")
                  nc.vector.tensor_scalar_mul(
                      tok_f[:], tx_bf[K_HOPS - 1][:, 0, 0:1], 0.0
                  )
                  tok_i = wp.tile([P, 1], I16, tag="toki", name=f"toki{rep}")
                  nc.vector.tensor_copy(tok_i[:], tok_f[:])
                  nc.vector.tensor_tensor(
                      out=idxw_sb[:],
                      in0=idxw_sb[:],
                      in1=tok_i[:, 0:1].to_broadcast([P, S // 16]),
                      op=mybir.AluOpType.add,
                  )
              if "warmcc" in ablate:
                  # dummy async AG at rep start: warms the collective path
                  # concurrently with hop-1 gathers; nobody waits on it
                  wbn = dp.tile([P, DPAD], TDT, name=f"wbn{rep}")
                  wpk = dp.tile(
                      [P * n_cores, DPAD], TDT, addr_space="Shared",
                      name=f"wpk{rep}",
                  )
                  nc.sync.dma_start(out=wbn[:], in_=tables[0][0:P, :])
                  _cc_allgather(nc, rg, wbn[:].opt(), wpk[:].opt())
              if "preag" in ablate and "ag" not in ablate:
                  # diagnostic: run the AGs up-front (inputs seeded from
                  # table0, no dependency on hop compute)
                  for k in range(1, K_HOPS - 1):
                      seed = wp.tile([P, BW], TDT, tag="seed", name=f"seed{rep}_{k}")
                      nc.sync.dma_start(out=seed[:], in_=tables[0][0:P, :BW])
                      for t in range(n_tiles):
                          nc.sync.dma_start(
                              out=bounces[k - 1][t * P : (t + 1) * P, :],
                              in_=seed[:],
                          )
                      _cc_allgather(
                          nc, rg, bounces[k - 1][:].opt(), packeds[k - 1][:].opt()
                      )
                      if TABLE_MODE == "direct":
                          pass
                      elif "smallwrite" in ablate:
                          nc.sync.dma_start(
                              out=tables[k][0:P, :BW], in_=packeds[k - 1][0:P]
                          )
                      else:
                          nc.sync.dma_start(
                              out=tables[k][:, :BW], in_=packeds[k - 1][:]
                          )
              for k in range(1, K_HOPS):
                tbl = tables[0] if "dangle" in ablate else tables[k - 1]
                if "ag1" in ablate and k == K_HOPS - 1:
                    tbl = tables[1]  # hop3 reuses hop2's table (timing diag)
                tbl_lo = tbl[0:lo_rows, :D]
                tbl_hi = tbl[lo_rows:npt, :D] if npt > lo_rows else None

                # gather calls -> gbuf slots keyed by chunk index
                gbuf_of_chunk = {}
                for qi, (h, c0, nch) in enumerate(calls):
                    gbuf = gp.tile(
                        [P, CALL_CHUNKS, D], TDT, tag="gbuf", bufs=10,
                        name=f"g{rep}_{k}_{c0}",
                    )
                    n_idx = nch * P
                    src = tbl_lo if h == 0 else tbl_hi
                    if "gather" not in ablate:
                        _dma_gather_rows(
                            nc.gpsimd,
                            out_ap=gbuf[:, :nch, :],
                            in_ap=src,
                            idxs_ap=idxw_use[:, c0 * 8 : c0 * 8 + n_idx // 16],
                            num_idxs=n_idx,
                            queue_num=qi % N_QUEUES,
                        )
                    for ci in range(c0, c0 + nch):
                        gbuf_of_chunk[ci] = (gbuf, ci - c0)

                # staircases, batched
                stair_of_chunk = {}
                for b0 in range(0, n_chunks, STAIR_BATCH):
                    nb = min(STAIR_BATCH, n_chunks - b0)
                    stair = stp.tile(
                        [P, STAIR_BATCH, W], TDT, tag="stair", bufs=3,
                        name=f"st{rep}_{k}_{b0}",
                    )
                    if "stair" not in ablate:
                        nc.vector.tensor_tensor(
                            out=stair[:, :nb, :],
                            in0=iota_sb[:, None, :W].to_broadcast([P, nb, W]),
                            in1=dv_sb[:, b0 : b0 + nb, None].to_broadcast([P, nb, W]),
                            op=mybir.AluOpType.is_equal,
                        )
                    for ci in range(b0, b0 + nb):
                        stair_of_chunk[ci] = (stair, ci - b0)

                # wave 1: low-half psums -> acc_sb (releases psum slots early)
                nwndw = P // W

                def do_win(t, h, w):
                    cs, ce = chunk_ranges[(t, h, w)]
                    if ce == cs or "matmul" in ablate:
                        return None
                    ps = pp.tile(
                        [W, D], F32, tag="pacc", bufs=4,
                        name=f"ps{rep}_{k}_{t}_{h}_{w}",
                    )
                    for ci in range(cs, ce):
                        gbuf, gcol = gbuf_of_chunk[ci]
                        stair, scol = stair_of_chunk[ci]
                        nc.tensor.matmul(
                            ps[:],
                            lhsT=stair[:, scol, :],
                            rhs=gbuf[:, gcol, :],
                            start=(ci == cs),
                            stop=(ci == ce - 1),
                        )
                    return ps

                has_lo = {}
                for t in range(n_tiles):
                    for w in range(nwndw):
                        ps = do_win(t, 0, w)
                        has_lo[(t, w)] = ps is not None
                        if ps is not None:
                            nc.scalar.copy(
                                acc_sb[w * W : (w + 1) * W, t, :], ps[:]
                            )

                # wave 2: high-half psums + reduce + recurrence
                for t in range(n_tiles):
                    red = wp.tile([P, D], F32, tag="red", name=f"red{rep}_{k}_{t}")
                    for w in range(nwndw):
                        ps_hi = do_win(t, 1, w)
                        sl = slice(w * W, (w + 1) * W)
                        if ps_hi is not None and has_lo[(t, w)]:
                            nc.vector.tensor_add(
                                red[sl, :], acc_sb[sl, t, :], ps_hi[:]
                            )
                        elif ps_hi is not None:
                            nc.vector.tensor_copy(red[sl, :], ps_hi[:])
                        elif has_lo[(t, w)]:
                            nc.vector.tensor_copy(red[sl, :], acc_sb[sl, t, :])
                        else:
                            nc.vector.memset(red[sl, :], 0.0)
                    src_red = red[:]

                    if k == 1:
                        dst = tx1_f[:, t, :]
                        nc.vector.tensor_scalar_mul(
                            dst, src_red, mdinv_sb[:, t : t + 1]
                        )
                    else:
                        dst = wp.tile(
                            [P, D], F32, tag="txtmp", name=f"tt{rep}_{k}_{t}"
                        )[:]
                        prev2 = tx_bf[0] if k == 2 else tx1_f
                        nc.vector.scalar_tensor_tensor(
                            out=dst,
                            in0=src_red,
                            scalar=m2dinv_sb[:, t : t + 1],
                            in1=prev2[:, t, :],
                            op0=mybir.AluOpType.mult,
                            op1=mybir.AluOpType.subtract,
                        )
                    nc.scalar.copy(tx_bf[k][:, t, :], dst)
                    if k < K_HOPS - 1:
                        nc.vector.tensor_scalar_mul(
                            h_all[:, t, :D], dst, dinv_sb[:, t : t + 1]
                        )
                if (
                    k < K_HOPS - 1
                    and "ag" not in ablate
                    and "preag" not in ablate
                    and not ("ag1" in ablate and k == K_HOPS - 2)
                ):
                    nc.sync.dma_start(
                        out=bounces[k - 1][:].rearrange("(t p) f -> p t f", p=P),
                        in_=h_all[:, :, :BW],
                    )
                    _cc_allgather(
                        nc, rg, bounces[k - 1][:].opt(), packeds[k - 1][:].opt()
                    )
                    if TABLE_MODE == "direct":
                        pass
                    elif "smallwrite" in ablate:
                        nc.sync.dma_start(
                            out=tables[k][0:P, :BW], in_=packeds[k - 1][0:P]
                        )
                    else:
                        nc.sync.dma_start(
                            out=tables[k][:, :BW], in_=packeds[k - 1][:]
                        )

            # ---- output: outT[:, tile] = sum_k W_k.T @ tx_k.T + bias ----
            for t in range(n_tiles):
                tts = []
                for k in range(K_HOPS):
                    tp = pp.tile([D, P], BF16, tag="tp", bufs=2, name=f"tp{t}_{k}")
                    nc.tensor.transpose(tp[:], tx_bf[k][:, t, :], ident[:])
                    tt = wp.tile([D, P], BF16, tag="tt", bufs=4, name=f"tt{t}_{k}")
                    nc.scalar.copy(tt[:], tp[:])
                    tts.append(tt)
                facc = pp.tile([D, P], F32, tag="facc", bufs=2, name=f"facc{t}")
                for k in range(K_HOPS):
                    nc.tensor.matmul(
                        facc[:],
                        lhsT=w_bf[:, k * D : (k + 1) * D],
                        rhs=tts[k][:],
                        start=(k == 0),
                        stop=(k == K_HOPS - 1),
                    )
                ot = wp.tile([D, P], BF16, tag="ot", bufs=3, name=f"ot{t}")
                nc.vector.tensor_scalar_add(ot[:], facc[:], bias_sb[:, 0:1])
                nc.sync.dma_start(out=t_out.ap()[:, t * P : (t + 1) * P], in_=ot[:])

    nc.compile()
    return nc


_CACHE = {}


def _get_cached(x, edge_index, n_nodes, n_cores):
    ei = np.asarray(edge_index)
    key = (int(ei[:, :1000].sum()) & 0xFFFFFFFF, ei.shape, n_nodes)
    pre = preprocess(x, edge_index, n_nodes, n_cores)
    if key not in _CACHE:
        _CACHE[key] = build_program(pre[0], n_cores)
    return pre, _CACHE[key]


def run(x, edge_index, weight, bias, n_nodes, n_cores, trace=False):
    (meta, table0, x_shards, dinv_pm, idxw_all, destvec_all), nc = _get_cached(
        x, edge_index, n_nodes, n_cores
    )
    w = np.ascontiguousarray(np.asarray(weight, dtype=np.float32))
    b = np.ascontiguousarray(np.asarray(bias, dtype=np.float32))
    in_maps = []
    for c in range(n_cores):
        in_maps.append(
            {
                "table0": table0,
                "x": x_shards[c],
                "dinv": dinv_pm[c],
                "idxw": idxw_all[c],
                "destvec": destvec_all[c],
                "w": w,
                "bias": b,
            }
        )
    res = bass_utils.run_bass_kernel_spmd(
        nc, in_maps, core_ids=list(range(n_cores)), trace=trace
    )
    npc = meta["npc"]
    npc_raw = meta["npc_raw"]
    out = np.concatenate(
        [
            res.results[c]["outT"].astype(np.float32).T[:npc_raw]
            for c in range(n_cores)
        ],
        axis=0,
    )
    return np.ascontiguousarray(out, dtype=np.float32), res, meta


def kernel(x, edge_index, weight, bias):
    out, _, _ = run(x, edge_index, weight, bias, N_NODES, N_CORES)
    return out



# revision 39
# speedup vs baseline: 6.5322x; 6.5322x over previous
"""ChebConv (K=4) message-passing kernel for 8 Trainium2 NeuronCores.

Architecture (1D graph partitioning by destination node):
  - 50000 nodes split contiguously into 8 shards of 6250, each padded to
    6272 = 49 tiles of 128 destinations.
  - Per hop, each core gathers the source rows of its ~100k edges from a
    replicated table (dinv-prescaled: table_k[v] = dinv[v] * tx_k[v], rows
    padded to a 256B stride) using the custom InstDMAGatherAnt (dma_gather)
    SWDGE instruction.  Indices are int16, so the table is addressed as a
    low half (rows < 32768) and a high half via two calls with different
    base APs.
  - The edge stream is ordered (half, dest-tile, 64-dest window, dest); each
    128-slot chunk is segment-reduced into its window's PSUM accumulator with
    a TensorE matmul against a one-hot "staircase" matrix
    (stair[slot, d] = 1 iff slot's dest-within-window == d), generated on the
    vector engine from iota==destvec.  Chunk padding slots have destvec -1.
  - Chebyshev recurrence tx_k = -2*dinv*red - tx_{k-2} on the vector engine;
    next hop's table rebuilt via a packed [npc, 96] AllGather of the rescaled
    shard plus a local restride DMA into the 256B-row-stride table.
  - out = sum_k tx_k @ W_k + bias via PE transposes + matmuls, written
    feature-major; the host strips padding.
"""

import os
import sys

for _p in ("/opt/trn_rl_repo", "/root/.axon_site/_ro/trn_rl_repo"):
    if os.path.isdir(_p) and _p not in sys.path:
        sys.path.insert(0, _p)
        break

import numpy as np

import concourse.bacc as bacc
import concourse.bass as bass
import concourse.mybir as mybir
import concourse.tile as tile
from concourse import bass_utils

F32 = mybir.dt.float32
BF16 = mybir.dt.bfloat16
I16 = mybir.dt.int16

USE_FP8 = True  # table/gather/stair dtype (halves AG + gather DMA bytes)
TDT = mybir.dt.float8e4 if USE_FP8 else BF16
NP_TDT = mybir.dt.np(TDT)
TSZ = 1 if USE_FP8 else 2

N_NODES = 50000
D = 96
DPAD = 256 // TSZ  # table row = 256B stride, in elements
K_HOPS = 4
N_CORES = 8
P = 128
LOBASE = 32768
CALL_CHUNKS = 8  # chunks per dma_gather call (1024 idxs — ucode ring cap;
# >1024-idx calls hang the device regardless of dynamic_dma_scratch_size)
DMA_SCRATCH = 65536  # headroom for 4 SWDGE queue rings
N_QUEUES = 4  # round-robin gather calls across SWDGE queues (~2.5x faster)
W = 128  # staircase window width (dests per psum accumulation group)
STAIR_BATCH = 32  # chunks per staircase-generation op


def _plan_sizes(n_nodes, n_cores):
    npc_raw = n_nodes // n_cores
    assert npc_raw * n_cores == n_nodes
    n_tiles = -(-npc_raw // P)
    npc = n_tiles * P
    return npc_raw, npc, n_tiles


def preprocess(x, edge_index, n_nodes, n_cores):
    npc_raw, npc, n_tiles = _plan_sizes(n_nodes, n_cores)
    npt = npc * n_cores
    n_pad = npc - npc_raw

    row = np.asarray(edge_index[0], dtype=np.int64)
    col = np.asarray(edge_index[1], dtype=np.int64)
    deg = np.bincount(row, minlength=n_nodes).astype(np.int64)
    dinv = np.zeros(n_nodes, dtype=np.float32)
    nz = deg > 0
    dinv[nz] = (1.0 / np.sqrt(deg[nz].astype(np.float64))).astype(np.float32)

    # pad-only remap: node v -> v + n_pad * (v // npc_raw)
    blk = np.arange(n_nodes) // npc_raw
    new_id = np.arange(n_nodes) + n_pad * blk

    x_new = np.zeros((npt, D), dtype=np.float32)
    x_new[new_id] = np.asarray(x, dtype=np.float32)
    dinv_new = np.zeros(npt, dtype=np.float32)
    dinv_new[new_id] = dinv

    # packed quantized full-graph table source; device restrides to 256B rows
    xq = np.ascontiguousarray((x_new * dinv_new[:, None]).astype(NP_TDT))

    row_new = new_id[row]
    col_new = new_id[col]
    core_of_edge = row_new // npc

    # ---- global chunk schedule (same for all cores) ----
    # chunks are grouped by (tile, half, 64-dest window) so the staircase
    # matrices are only 64 wide
    d_loc_all = row_new % npc
    t_all = d_loc_all // P
    w_all = (d_loc_all % P) // W  # window within tile
    h_all = (col_new >= LOBASE).astype(np.int64)
    nw = P // W
    counts = np.zeros((n_cores, n_tiles, 2, nw), dtype=np.int64)
    np.add.at(counts, (core_of_edge, t_all, h_all, w_all), 1)
    n_ch = -(-counts.max(axis=0) // P)  # [n_tiles, 2, nw]

    # stream order: all lo (t, w ascending), then all hi
    chunk_base = np.zeros((n_tiles, 2, nw), dtype=np.int64)
    pos = 0
    order_th = [(t, 0, w) for t in range(n_tiles) for w in range(nw)] + [
        (t, 1, w) for t in range(n_tiles) for w in range(nw)
    ]
    chunk_ranges = {}
    for t, h, w in order_th:
        chunk_base[t, h, w] = pos
        chunk_ranges[(t, h, w)] = (pos, pos + int(n_ch[t, h, w]))
        pos += int(n_ch[t, h, w])
    n_chunks = pos
    n_lo_chunks = int(n_ch[:, 0, :].sum())
    S = n_chunks * P  # total slots

    # call plan: contiguous chunk runs, single half, <= CALL_CHUNKS
    calls = []  # (half, chunk_start, n_chunks)
    for h, lo_, hi_ in ((0, 0, n_lo_chunks), (1, n_lo_chunks, n_chunks)):
        c0 = lo_
        while c0 < hi_:
            n = min(CALL_CHUNKS, hi_ - c0)
            calls.append((h, c0, n))
            c0 += n

    # ---- per-core streams ----
    idxw_all = []
    destvec_all = []
    x_shards = []
    dinv_pm = []
    for c in range(n_cores):
        m = core_of_edge == c
        d_loc = d_loc_all[m]
        hh = h_all[m]
        cn = col_new[m]
        nw = P // W
        g_un = (hh * n_tiles + d_loc // P) * nw + (d_loc % P) // W
        order = np.lexsort((d_loc, g_un))
        d_s = d_loc[order]
        h_s = hh[order]
        c_s = cn[order] - h_s * LOBASE
        t_s = d_s // P
        w_s = (d_s % P) // W
        g_s = g_un[order]  # group id in stream order
        gcnt = np.bincount(g_s, minlength=2 * n_tiles * nw)
        gstart = np.concatenate([[0], np.cumsum(gcnt)])[:-1]
        pos_in_g = np.arange(len(d_s)) - gstart[g_s]
        base_slots = chunk_base[t_s, h_s, w_s] * P
        slot = base_slots + pos_in_g

        idx_stream = np.zeros(S, dtype=np.int16)
        destvec = np.full(S, -1.0, dtype=mybir.dt.np(BF16))
        idx_stream[slot] = c_s.astype(np.int16)
        destvec[slot] = (d_s % W).astype(mybir.dt.np(BF16))

        # wrapped idx layout for dma_gather: [16, S//16]; the device
        # replicates to [128, S//16] (8x less input staging)
        w16 = idx_stream.reshape(S // 16, 16).T  # [16, S//16]
        idxw_all.append(np.ascontiguousarray(w16))
        # destvec partition-major per chunk: [128, n_chunks]
        destvec_all.append(
            np.ascontiguousarray(destvec.reshape(n_chunks, P).T)
        )
        x_shards.append(np.ascontiguousarray(x_new[c * npc : (c + 1) * npc]))
        dinv_pm.append(
            np.ascontiguousarray(
                dinv_new[c * npc : (c + 1) * npc].reshape(n_tiles, P).T
            )
        )

    meta = dict(
        npc_raw=npc_raw,
        npc=npc,
        n_tiles=n_tiles,
        npt=npt,
        S=S,
        n_chunks=n_chunks,
        n_lo_chunks=n_lo_chunks,
        chunk_ranges=chunk_ranges,
        calls=calls,
        new_id=new_id,
    )
    return meta, xq, x_shards, dinv_pm, idxw_all, destvec_all


CC_ENGINE = "gpsimd"  # walrus' checkValidEngines only allows Pool for ccs
# "copy":     AG full-width [npt, DPAD] Shared packed, plain copy to Local
# "direct":   AG full-width Shared packed IS the gather table (no copy)
# "packed96": AG [npt, D] packed (2.7x fewer cc bytes), strided restride
TABLE_MODE = os.environ.get("TABLE_MODE", "direct")


def _cc_allgather(nc, rg, in_ap, out_ap):
    eng = getattr(nc, CC_ENGINE)
    bass.BassGpSimd.collective_compute(
        eng,
        "AllGather",
        mybir.AluOpType.bypass,
        replica_groups=rg,
        ins=[in_ap],
        outs=[out_ap],
    )


def _dma_gather_rows(g, out_ap, in_ap, idxs_ap, num_idxs, queue_num=0):
    """bass.dma_gather minus the %256 payload assert (non-transpose HBM
    path): gathers D elems (D*TSZ bytes) per index from 256B-strided rows."""
    import concourse.ap_utils as ap_utils

    elem_size, elem_step = D, DPAD
    assert idxs_ap.dtype == I16
    assert in_ap.ap[0][0] == elem_step
    assert in_ap.ap[-1][1] == out_ap.ap[-1][1] == elem_size
    assert ap_utils.ap_is_contiguous(out_ap.ap[1:])
    assert ap_utils.ap_is_contiguous(idxs_ap.ap[1:])
    assert out_ap.ap[0][1] * out_ap.ap[1][1] == num_idxs
    stride_bytes_256 = (elem_step * TSZ) // 256
    _in_ap = g.lower_ap_dma(in_ap, for_custom_bir_dma=True)
    _idxs_ap = g.lower_ap(idxs_ap)
    _out_ap = g.lower_ap(out_ap)
    return g.add_instruction(
        mybir.InstDMAGatherAnt(
            name=g.bass.get_next_instruction_name(),
            ins=[*_in_ap, _idxs_ap, g.lower_val_access(g.to_reg(num_idxs))],
            outs=[_out_ap],
            transpose=False,
            num_idxs=num_idxs,
            elem_size=elem_size,
            stride_bytes_256=stride_bytes_256,
            gen_mode=0,
            single_packet=True,
            queue_num=queue_num,
            sbuf_tokens_per_rank=0,
            sbuf_free_dim_per_rank=0,
            sbuf_free_dim_pad_per_rank=0,
            sbuf_byte_offset=0,
        )
    )


def build_program(meta, n_cores, repeat=1, ablate=frozenset()):
    npc = meta["npc"]
    n_tiles = meta["n_tiles"]
    npt = meta["npt"]
    S = meta["S"]
    n_chunks = meta["n_chunks"]
    chunk_ranges = meta["chunk_ranges"]
    calls = meta["calls"]
    lo_rows = min(LOBASE, npt)

    nc = bacc.Bacc(
        "TRN2",
        target_bir_lowering=False,
        debug=False,
        num_devices=n_cores,
        dynamic_dma_scratch_size=DMA_SCRATCH,
        num_swdge_queues=N_QUEUES,
    )
    t_xq = nc.dram_tensor("xq", [npt, D], TDT, kind="ExternalInput")
    t_x = nc.dram_tensor("x", [npc, D], F32, kind="ExternalInput")
    t_dinv = nc.dram_tensor("dinv", [P, n_tiles], F32, kind="ExternalInput")
    t_idxw = nc.dram_tensor("idxw", [16, S // 16], I16, kind="ExternalInput")
    t_dv = nc.dram_tensor("destvec", [P, n_chunks], BF16, kind="ExternalInput")
    t_w = nc.dram_tensor("w", [K_HOPS, D, D], F32, kind="ExternalInput")
    t_b = nc.dram_tensor("bias", [D], F32, kind="ExternalInput")
    t_out = nc.dram_tensor("outT", [D, npc], BF16, kind="ExternalOutput")

    rg = [list(range(n_cores))]

    with tile.TileContext(nc) as tc:
        with (
            tc.tile_pool(name="persist", bufs=1) as sb,
            tc.tile_pool(name="gather", bufs=3) as gp,
            tc.tile_pool(name="stair", bufs=3) as stp,
            tc.tile_pool(name="work", bufs=3) as wp,
            tc.tile_pool(name="dram", bufs=1, space="DRAM") as dp,
            tc.tile_pool(name="psum", bufs=1, space="PSUM") as pp,
        ):
            # ---- persistent loads ----
            idxw_sb = sb.tile([P, S // 16], I16)
            for rk in range(P // 16):
                nc.sync.dma_start(
                    out=idxw_sb[rk * 16 : (rk + 1) * 16, :], in_=t_idxw.ap()
                )
            # device-side table0: restride packed xq into 256B gather rows
            table0_dev = dp.tile([npt, DPAD], TDT, name="table0dev")
            nc.sync.dma_start(out=table0_dev[:][:, 0:D], in_=t_xq.ap())
            dv_sb = sb.tile([P, n_chunks], BF16)
            nc.sync.dma_start(out=dv_sb[:], in_=t_dv.ap())
            dinv_sb = sb.tile([P, n_tiles], F32)
            nc.sync.dma_start(out=dinv_sb[:], in_=t_dinv.ap())
            mdinv_sb = sb.tile([P, n_tiles], F32)
            nc.vector.tensor_scalar_mul(mdinv_sb[:], dinv_sb[:], -1.0)
            m2dinv_sb = sb.tile([P, n_tiles], F32)
            nc.vector.tensor_scalar_mul(m2dinv_sb[:], dinv_sb[:], -2.0)

            w_f32 = wp.tile([D, K_HOPS * D], F32, tag="wf")
            for k in range(K_HOPS):
                nc.sync.dma_start(out=w_f32[:, k * D : (k + 1) * D], in_=t_w.ap()[k])
            w_bf = sb.tile([D, K_HOPS * D], BF16)
            nc.vector.tensor_copy(w_bf[:], w_f32[:])
            bias_sb = sb.tile([D, 1], F32)
            nc.sync.dma_start(out=bias_sb[:], in_=t_b.ap()[:, None])

            from concourse.masks import make_identity

            ident = sb.tile([P, P], BF16)
            make_identity(nc, ident[:])

            iota_sb = sb.tile([P, P], BF16)
            nc.gpsimd.iota(
                iota_sb[:],
                pattern=[[1, P]],
                base=0,
                channel_multiplier=0,
                allow_small_or_imprecise_dtypes=True,
            )

            # tx buffers
            tx_bf = []
            for k in range(K_HOPS):
                txb = sb.tile([P, n_tiles, D], BF16, name=f"tx_bf{k}")
                tx_bf.append(txb)
            tx1_f = sb.tile([P, n_tiles, D], F32)
            acc_sb = sb.tile([P, n_tiles, D], F32)
            # h_all rows padded to the 256B table stride so the AllGather
            # lands directly in gatherable table layout (no restride DMA)
            h_all = sb.tile([P, n_tiles, DPAD], TDT, name="h_all")
            nc.vector.memset(h_all[:], 0.0)

            # x -> tx_bf[0] via acc_sb (acc_sb is scratch until hop 1)
            nc.sync.dma_start(
                out=acc_sb[:], in_=t_x.ap().rearrange("(t p) f -> p t f", p=P)
            )
            nc.vector.tensor_copy(tx_bf[0][:], acc_sb[:])

            # DRAM tables / bounce buffers (per repeat: Shared tiles allow
            # only a single writer).
            BW = D if TABLE_MODE == "packed96" else DPAD  # AG row width
            tables_r = []
            bounces_r = []
            packed_r = []
            for rep in range(repeat):
                tables = [table0_dev[:]]
                bounces = []
                packeds = []
                for k in range(1, K_HOPS - 1):
                    pk = dp.tile(
                        [npt, BW], TDT, addr_space="Shared",
                        name=f"packed{rep}_{k}",
                    )
                    if TABLE_MODE == "direct":
                        tables.append(pk[:])
                    else:
                        tb = dp.tile([npt, DPAD], TDT, name=f"table{rep}_{k}")
                        tables.append(tb[:])
                    bn = dp.tile([npc, BW], TDT, name=f"bounce{rep}_{k}")
                    bounces.append(bn)
                    packeds.append(pk)
                tables_r.append(tables)
                bounces_r.append(bounces)
                packed_r.append(packeds)

            # ---- hops ----
            for rep in range(repeat):
              tables = tables_r[rep]
              bounces = bounces_r[rep]
              packeds = packed_r[rep]
              idxw_use = idxw_sb
              if "serial" in ablate and rep > 0:
                  # chain rep r's gathers on rep r-1's final tx state so the
                  # repeat-slope measures single-invocation latency (~2us
                  # pollution per rep)
                  tok_f = wp.tile([P, 1], F32, tag="tok", name=f"tok{rep}")
                  nc.vector.tensor_scalar_mul(
                      tok_f[:], tx_bf[K_HOPS - 1][:, 0, 0:1], 0.0
                  )
                  tok_i = wp.tile([P, 1], I16, tag="toki", name=f"toki{rep}")
                  nc.vector.tensor_copy(tok_i[:], tok_f[:])
                  nc.vector.tensor_tensor(
                      out=idxw_sb[:],
                      in0=idxw_sb[:],
                      in1=tok_i[:, 0:1].to_broadcast([P, S // 16]),
                      op=mybir.AluOpType.add,
                  )
              if "warmcc" in ablate:
                  # dummy async AG at rep start: warms the collective path
                  # concurrently with hop-1 gathers; nobody waits on it
                  wbn = dp.tile([P, DPAD], TDT, name=f"wbn{rep}")
                  wpk = dp.tile(
                      [P * n_cores, DPAD], TDT, addr_space="Shared",
                      name=f"wpk{rep}",
                  )
                  nc.sync.dma_start(out=wbn[:], in_=tables[0][0:P, :])
                  _cc_allgather(nc, rg, wbn[:].opt(), wpk[:].opt())
              if "preag" in ablate and "ag" not in ablate:
                  # diagnostic: run the AGs up-front (inputs seeded from
                  # table0, no dependency on hop compute)
                  for k in range(1, K_HOPS - 1):
                      seed = wp.tile([P, BW], TDT, tag="seed", name=f"seed{rep}_{k}")
                      nc.sync.dma_start(out=seed[:], in_=tables[0][0:P, :BW])
                      for t in range(n_tiles):
                          nc.sync.dma_start(
                              out=bounces[k - 1][t * P : (t + 1) * P, :],
                              in_=seed[:],
                          )
                      _cc_allgather(
                          nc, rg, bounces[k - 1][:].opt(), packeds[k - 1][:].opt()
                      )
                      if TABLE_MODE == "direct":
                          pass
                      elif "smallwrite" in ablate:
                          nc.sync.dma_start(
                              out=tables[k][0:P, :BW], in_=packeds[k - 1][0:P]
                          )
                      else:
                          nc.sync.dma_start(
                              out=tables[k][:, :BW], in_=packeds[k - 1][:]
                          )
              for k in range(1, K_HOPS):
                tbl = tables[0] if "dangle" in ablate else tables[k - 1]
                if "ag1" in ablate and k == K_HOPS - 1:
                    tbl = tables[1]  # hop3 reuses hop2's table (timing diag)
                tbl_lo = tbl[0:lo_rows, :D]
                tbl_hi = tbl[lo_rows:npt, :D] if npt > lo_rows else None

                # gather calls -> gbuf slots keyed by chunk index
                gbuf_of_chunk = {}
                for qi, (h, c0, nch) in enumerate(calls):
                    gbuf = gp.tile(
                        [P, CALL_CHUNKS, D], TDT, tag="gbuf", bufs=10,
                        name=f"g{rep}_{k}_{c0}",
                    )
                    n_idx = nch * P
                    src = tbl_lo if h == 0 else tbl_hi
                    if "gather" not in ablate:
                        _dma_gather_rows(
                            nc.gpsimd,
                            out_ap=gbuf[:, :nch, :],
                            in_ap=src,
                            idxs_ap=idxw_use[:, c0 * 8 : c0 * 8 + n_idx // 16],
                            num_idxs=n_idx,
                            queue_num=qi % N_QUEUES,
                        )
                    for ci in range(c0, c0 + nch):
                        gbuf_of_chunk[ci] = (gbuf, ci - c0)

                # staircases, batched
                stair_of_chunk = {}
                for b0 in range(0, n_chunks, STAIR_BATCH):
                    nb = min(STAIR_BATCH, n_chunks - b0)
                    stair = stp.tile(
                        [P, STAIR_BATCH, W], TDT, tag="stair", bufs=3,
                        name=f"st{rep}_{k}_{b0}",
                    )
                    if "stair" not in ablate:
                        nc.vector.tensor_tensor(
                            out=stair[:, :nb, :],
                            in0=iota_sb[:, None, :W].to_broadcast([P, nb, W]),
                            in1=dv_sb[:, b0 : b0 + nb, None].to_broadcast([P, nb, W]),
                            op=mybir.AluOpType.is_equal,
                        )
                    for ci in range(b0, b0 + nb):
                        stair_of_chunk[ci] = (stair, ci - b0)

                # wave 1: low-half psums -> acc_sb (releases psum slots early)
                nwndw = P // W

                def do_win(t, h, w):
                    cs, ce = chunk_ranges[(t, h, w)]
                    if ce == cs or "matmul" in ablate:
                        return None
                    ps = pp.tile(
                        [W, D], F32, tag="pacc", bufs=4,
                        name=f"ps{rep}_{k}_{t}_{h}_{w}",
                    )
                    for ci in range(cs, ce):
                        gbuf, gcol = gbuf_of_chunk[ci]
                        stair, scol = stair_of_chunk[ci]
                        nc.tensor.matmul(
                            ps[:],
                            lhsT=stair[:, scol, :],
                            rhs=gbuf[:, gcol, :],
                            start=(ci == cs),
                            stop=(ci == ce - 1),
                        )
                    return ps

                has_lo = {}
                for t in range(n_tiles):
                    for w in range(nwndw):
                        ps = do_win(t, 0, w)
                        has_lo[(t, w)] = ps is not None
                        if ps is not None:
                            nc.scalar.copy(
                                acc_sb[w * W : (w + 1) * W, t, :], ps[:]
                            )

                # wave 2: high-half psums + reduce + recurrence
                for t in range(n_tiles):
                    red = wp.tile([P, D], F32, tag="red", name=f"red{rep}_{k}_{t}")
                    for w in range(nwndw):
                        ps_hi = do_win(t, 1, w)
                        sl = slice(w * W, (w + 1) * W)
                        if ps_hi is not None and has_lo[(t, w)]:
                            nc.vector.tensor_add(
                                red[sl, :], acc_sb[sl, t, :], ps_hi[:]
                            )
                        elif ps_hi is not None:
                            nc.vector.tensor_copy(red[sl, :], ps_hi[:])
                        elif has_lo[(t, w)]:
                            nc.vector.tensor_copy(red[sl, :], acc_sb[sl, t, :])
                        else:
                            nc.vector.memset(red[sl, :], 0.0)
                    src_red = red[:]

                    if k == 1:
                        dst = tx1_f[:, t, :]
                        nc.vector.tensor_scalar_mul(
                            dst, src_red, mdinv_sb[:, t : t + 1]
                        )
                    else:
                        dst = wp.tile(
                            [P, D], F32, tag="txtmp", name=f"tt{rep}_{k}_{t}"
                        )[:]
                        prev2 = tx_bf[0] if k == 2 else tx1_f
                        nc.vector.scalar_tensor_tensor(
                            out=dst,
                            in0=src_red,
                            scalar=m2dinv_sb[:, t : t + 1],
                            in1=prev2[:, t, :],
                            op0=mybir.AluOpType.mult,
                            op1=mybir.AluOpType.subtract,
                        )
                    nc.scalar.copy(tx_bf[k][:, t, :], dst)
                    if k < K_HOPS - 1:
                        nc.vector.tensor_scalar_mul(
                            h_all[:, t, :D], dst, dinv_sb[:, t : t + 1]
                        )
                if (
                    k < K_HOPS - 1
                    and "ag" not in ablate
                    and "preag" not in ablate
                    and not ("ag1" in ablate and k == K_HOPS - 2)
                ):
                    nc.sync.dma_start(
                        out=bounces[k - 1][:].rearrange("(t p) f -> p t f", p=P),
                        in_=h_all[:, :, :BW],
                    )
                    _cc_allgather(
                        nc, rg, bounces[k - 1][:].opt(), packeds[k - 1][:].opt()
                    )
                    if TABLE_MODE == "direct":
                        pass
                    elif "smallwrite" in ablate:
                        nc.sync.dma_start(
                            out=tables[k][0:P, :BW], in_=packeds[k - 1][0:P]
                        )
                    else:
                        nc.sync.dma_start(
                            out=tables[k][:, :BW], in_=packeds[k - 1][:]
                        )

            # ---- output: outT[:, tile] = sum_k W_k.T @ tx_k.T + bias ----
            for t in range(n_tiles):
                tts = []
                for k in range(K_HOPS):
                    tp = pp.tile([D, P], BF16, tag="tp", bufs=2, name=f"tp{t}_{k}")
                    nc.tensor.transpose(tp[:], tx_bf[k][:, t, :], ident[:])
                    tt = wp.tile([D, P], BF16, tag="tt", bufs=4, name=f"tt{t}_{k}")
                    nc.scalar.copy(tt[:], tp[:])
                    tts.append(tt)
                facc = pp.tile([D, P], F32, tag="facc", bufs=2, name=f"facc{t}")
                for k in range(K_HOPS):
                    nc.tensor.matmul(
                        facc[:],
                        lhsT=w_bf[:, k * D : (k + 1) * D],
                        rhs=tts[k][:],
                        start=(k == 0),
                        stop=(k == K_HOPS - 1),
                    )
                ot = wp.tile([D, P], BF16, tag="ot", bufs=3, name=f"ot{t}")
                nc.vector.tensor_scalar_add(ot[:], facc[:], bias_sb[:, 0:1])
                nc.sync.dma_start(out=t_out.ap()[:, t * P : (t + 1) * P], in_=ot[:])

    nc.compile()
    return nc


_CACHE = {}


def _get_cached(x, edge_index, n_nodes, n_cores):
    ei = np.asarray(edge_index)
    key = (int(ei[:, :1000].sum()) & 0xFFFFFFFF, ei.shape, n_nodes)
    pre = preprocess(x, edge_index, n_nodes, n_cores)
    if key not in _CACHE:
        _CACHE[key] = build_program(pre[0], n_cores)
    return pre, _CACHE[key]


def run(x, edge_index, weight, bias, n_nodes, n_cores, trace=False):
    (meta, xq, x_shards, dinv_pm, idxw_all, destvec_all), nc = _get_cached(
        x, edge_index, n_nodes, n_cores
    )
    w = np.ascontiguousarray(np.asarray(weight, dtype=np.float32))
    b = np.ascontiguousarray(np.asarray(bias, dtype=np.float32))
    in_maps = []
    for c in range(n_cores):
        in_maps.append(
            {
                "xq": xq,
                "x": x_shards[c],
                "dinv": dinv_pm[c],
                "idxw": idxw_all[c],
                "destvec": destvec_all[c],
                "w": w,
                "bias": b,
            }
        )
    res = bass_utils.run_bass_kernel_spmd(
        nc, in_maps, core_ids=list(range(n_cores)), trace=trace
    )
    npc = meta["npc"]
    npc_raw = meta["npc_raw"]
    out = np.concatenate(
        [
            res.results[c]["outT"].astype(np.float32).T[:npc_raw]
            for c in range(n_cores)
        ],
        axis=0,
    )
    return np.ascontiguousarray(out, dtype=np.float32), res, meta


def kernel(x, edge_index, weight, bias):
    out, _, _ = run(x, edge_index, weight, bias, N_NODES, N_CORES)
    return out

